# revision 1
# baseline (speedup 1.0000x reference)
"""CrossViewSwapAttention Trainium2 kernel.

Problem (per full input):
  q (1,6,8,8,16,16,128), k/v (1,6,8,8,6,6,128), skip (1,8,8,16,16,128).
  Per window (x,y) of the 8x8 grid: LayerNorm+Linear projections of q/k/v
  tokens, 4-head attention (1536 queries x 216 keys, head dim 32), output
  projection, mean over the 6 views, plus skip.

Sharding: the window-grid x axis (8) across the 8 NeuronCores; each core
handles 8 windows (one grid row). Weights replicated.

Per-core layout strategy (all "T" tensors are feature-major, i.e.
[feature/k on partitions, tokens on free]):
  - load x natural [tok,128], LN stats via bn_stats, normalize (f32->bf16)
  - PE-transpose x_hat -> x_hatT, project with W' = diag(g) @ W (bias terms
    folded: qk-side bias added per-partition on the projected output; v-side
    bias folded into the output-projection bias via sum(att)=1)
  - scores computed transposed, dotT[k, q], per head via PE row-tiling
    (K=32); exp on ACT straight out of PSUM (no max subtraction: scores are
    O(0.1) so exp is safe and softmax is shift-free mathematically)
  - denominator via col-tiled ones-matmul (replicated per 32-row head
    group), reciprocal_approx_fast, fold into the PSUM->SBUF copy of att@v
  - output projection accumulates the 6 view blocks directly in PSUM
    (mean over views), bias via the copy, PE-transpose back, add skip.
"""

import numpy as np

import concourse.bass as bass
import concourse.tile as tile
from concourse import mybir
from concourse.bass_utils import run_bass_kernel_spmd
from concourse.masks import make_identity

F32 = mybir.dt.float32
BF16 = mybir.dt.bfloat16

HEADS = 4
DIM_HEAD = 32
D = 128
INNER = HEADS * DIM_HEAD  # 128
NWIN = 8          # windows per core (grid y)
NVIEW = 6
QTOK = NVIEW * 256   # 1536 queries per window
KTOK = NVIEW * 36    # 216 keys per window
KCH = 108            # k-token chunk (2 chunks of 108 = 216)
QB = 512             # q block for matmuls
SCALE = DIM_HEAD ** -0.5
EPS = 1e-5

# walrus in this container rejects >1 sync-wait per instruction
MAXW = 1


def _split_waits(nc, maxw=MAXW):
    """Split multi-sem waits onto same-engine Drain instructions inserted
    immediately before the owning instruction (engine-order equivalent)."""
    for f in nc.m.functions:
        for bb in f.blocks:
            insts = list(bb.instructions)
            newl, changed = [], False
            for inst in insts:
                si = inst.sync_info
                if si is not None and len(si.on_wait) > maxw:
                    waits = list(si.on_wait)
                    changed = True
                    k = 0
                    while len(waits) > maxw:
                        chunk, waits = waits[:maxw], waits[maxw:]
                        newl.append(mybir.InstDrain(
                            name=f"{inst.name}-wsplit{k}",
                            engine=inst.engine,
                            sync_info=mybir.SyncInfo(on_wait=chunk, on_update=[]),
                        ))
                        k += 1
                    inst.sync_info = mybir.SyncInfo(
                        on_wait=waits, on_update=list(si.on_update))
                newl.append(inst)
            if changed:
                bb.instructions = newl


def build_nc():
    nc = bass.Bass()
    AF = mybir.ActivationFunctionType

    q_t = nc.dram_tensor("q", (NVIEW, NWIN, 16, 16, D), F32, kind="ExternalInput")
    k_t = nc.dram_tensor("k", (NVIEW, NWIN, 6, 6, D), F32, kind="ExternalInput")
    v_t = nc.dram_tensor("v", (NVIEW, NWIN, 6, 6, D), F32, kind="ExternalInput")
    skip_t = nc.dram_tensor("skip", (NWIN, 16, 16, D), F32, kind="ExternalInput")
    # Wq, Wk, Wv, Wp stacked; params packed column-wise (see kernel()).
    w_t = nc.dram_tensor("wstack", (4, D, D), F32, kind="ExternalInput")
    p_t = nc.dram_tensor("pstack", (D, 10), F32, kind="ExternalInput")
    out_t = nc.dram_tensor("out", (NWIN, 16, 16, D), F32, kind="ExternalOutput")

    from contextlib import ExitStack
    with tile.TileContext(nc) as tc, ExitStack() as ctx:
        cpool = ctx.enter_context(tc.tile_pool(name="consts", bufs=1))
        sb = ctx.enter_context(tc.tile_pool(name="sb", bufs=2))
        etp = ctx.enter_context(tc.tile_pool(name="et", bufs=16))
        dotp = ctx.enter_context(tc.tile_pool(name="dot", bufs=2, space="PSUM"))
        mps = ctx.enter_context(tc.tile_pool(name="mps", bufs=2, space="PSUM"))

        # ---------------- constants / weight prep ----------------
        wraw = cpool.tile([D, 4, D], F32)
        nc.sync.dma_start(out=wraw, in_=w_t.rearrange("i d o -> d i o"))
        ptile = cpool.tile([D, 10], F32)
        nc.sync.dma_start(out=ptile, in_=p_t[:, :])

        wq_b = cpool.tile([D, D], BF16)
        wk_b = cpool.tile([D, D], BF16)
        wv_b = cpool.tile([D, D], BF16)
        wp_b = cpool.tile([D, D], BF16)
        nc.vector.tensor_scalar_mul(out=wq_b, in0=wraw[:, 0, :], scalar1=ptile[:, 0:1])
        nc.vector.tensor_scalar_mul(out=wk_b, in0=wraw[:, 1, :], scalar1=ptile[:, 2:3])
        nc.vector.tensor_scalar_mul(out=wv_b, in0=wraw[:, 2, :], scalar1=ptile[:, 4:5])
        nc.vector.tensor_copy(wp_b, wraw[:, 3, :])

        # bias columns: bwq = Wq^T bq_ln + bq ; bwk likewise ; bwv = Wv^T bv_ln + bv
        # bpe = bp + Wp^T bwv   (v-side bias folded through attention)
        bwq = cpool.tile([D, 1], F32)
        bwk = cpool.tile([D, 1], F32)
        bwv = cpool.tile([D, 1], F32)
        bpe = cpool.tile([D, 1], F32)
        bps = mps.tile([D, 4], F32, tag="misc")
        nc.tensor.matmul(bps[:, 0:1], wraw[:, 0, :], ptile[:, 1:2])
        nc.tensor.matmul(bps[:, 1:2], wraw[:, 1, :], ptile[:, 3:4])
        nc.tensor.matmul(bps[:, 2:3], wraw[:, 2, :], ptile[:, 5:6])
        nc.vector.tensor_add(out=bwq, in0=bps[:, 0:1], in1=ptile[:, 6:7])
        nc.vector.tensor_add(out=bwk, in0=bps[:, 1:2], in1=ptile[:, 7:8])
        nc.vector.tensor_add(out=bwv, in0=bps[:, 2:3], in1=ptile[:, 8:9])
        bps2 = mps.tile([D, 1], F32, tag="misc")
        nc.tensor.matmul(bps2, wraw[:, 3, :], bwv[:, 0:1])
        nc.vector.tensor_add(out=bpe, in0=bps2, in1=ptile[:, 9:10])

        id_bf = cpool.tile([D, D], BF16)
        id_f32 = cpool.tile([D, D], F32)
        make_identity(nc, id_bf)
        make_identity(nc, id_f32)
        ones_bf = cpool.tile([D, DIM_HEAD], BF16)
        nc.vector.memset(ones_bf, 1.0)
        eps_c = cpool.tile([D, 1], F32)
        nc.vector.memset(eps_c, EPS)

        # ---------------- per-window pipeline ----------------
        for w in range(NWIN):
            # ---- load natural-layout inputs
            xq = sb.tile([128, 12, D], F32, tag="xq")
            for n in range(NVIEW):
                nc.sync.dma_start(
                    out=xq[:, 2 * n:2 * n + 2, :],
                    in_=q_t[n, w].rearrange("a b d -> (a b) d")
                               .rearrange("(c p) d -> p c d", p=128))
            xk = sb.tile([KCH, 2, D], F32, tag="xk")
            xv = sb.tile([KCH, 2, D], F32, tag="xv")
            for n in range(NVIEW):
                c, m = divmod(n, 3)
                nc.sync.dma_start(
                    out=xk[36 * m:36 * m + 36, c, :],
                    in_=k_t[n, w].rearrange("a b d -> (a b) d"))
                nc.sync.dma_start(
                    out=xv[36 * m:36 * m + 36, c, :],
                    in_=v_t[n, w].rearrange("a b d -> (a b) d"))

            # ---- LN stats (mean/var per token)
            st_q = sb.tile([128, 12, 6], F32, tag="stq")
            for j in range(12):
                nc.vector.bn_stats(out=st_q[:, j, :], in_=xq[:, j, :])
            st_k = sb.tile([KCH, 2, 6], F32, tag="stk")
            st_v = sb.tile([KCH, 2, 6], F32, tag="stv")
            for c in range(2):
                nc.vector.bn_stats(out=st_k[:, c, :], in_=xk[:, c, :])
                nc.vector.bn_stats(out=st_v[:, c, :], in_=xv[:, c, :])

            mv = sb.tile([128, 16, 2], F32, tag="mv")
            nc.vector.memset(mv, 1.0)
            for j in range(12):
                nc.vector.bn_aggr(out=mv[:, j, :], in_=st_q[:, j, :])
            for c in range(2):
                nc.vector.bn_aggr(out=mv[:KCH, 12 + c, :], in_=st_k[:, c, :])
                nc.vector.bn_aggr(out=mv[:KCH, 14 + c, :], in_=st_v[:, c, :])

            # rs = (var+eps)^-1/2 = exp(-0.5*ln(var+eps)); stays in the
            # natural_log_exp ACT table set (same set as softmax exp).
            lnv = sb.tile([128, 16], F32, tag="lnv")
            rs = sb.tile([128, 16], F32, tag="rs")
            nc.scalar.activation(out=lnv, in_=mv[:, :, 1], func=AF.Ln, bias=eps_c[:, 0:1])
            nc.scalar.activation(out=rs, in_=lnv, func=AF.Exp, scale=-0.5)

            # ---- normalize -> bf16 (gamma/beta folded into W'/bias)
            xh_q = sb.tile([128, 12, D], BF16, tag="xhq")
            for j in range(12):
                nc.vector.tensor_scalar(
                    out=xh_q[:, j, :], in0=xq[:, j, :],
                    scalar1=mv[:, j, 0:1], scalar2=rs[:, j:j + 1],
                    op0=mybir.AluOpType.subtract, op1=mybir.AluOpType.mult)
            xh_k = sb.tile([KCH, 2, D], BF16, tag="xhk")
            xh_v = sb.tile([KCH, 2, D], BF16, tag="xhv")
            for c in range(2):
                nc.vector.tensor_scalar(
                    out=xh_k[:, c, :], in0=xk[:, c, :],
                    scalar1=mv[:KCH, 12 + c, 0:1], scalar2=rs[:KCH, 12 + c:13 + c],
                    op0=mybir.AluOpType.subtract, op1=mybir.AluOpType.mult)
                nc.vector.tensor_scalar(
                    out=xh_v[:, c, :], in0=xv[:, c, :],
                    scalar1=mv[:KCH, 14 + c, 0:1], scalar2=rs[:KCH, 14 + c:15 + c],
                    op0=mybir.AluOpType.subtract, op1=mybir.AluOpType.mult)

            # ---- transpose to feature-major via PE
            xhqT = sb.tile([128, QTOK], BF16, tag="xhqT")
            for g in range(3):
                tp = mps.tile([128, 512], BF16, tag="misc")
                for i in range(4):
                    j = 4 * g + i
                    nc.tensor.transpose(tp[:, 128 * i:128 * i + 128],
                                        xh_q[:, j, :], id_bf)
                nc.vector.tensor_copy(xhqT[:, 512 * g:512 * g + 512], tp)
            xhkT = sb.tile([128, KTOK], BF16, tag="xhkT")
            xhvT = sb.tile([128, KTOK], BF16, tag="xhvT")
            for src, dst in ((xh_k, xhkT), (xh_v, xhvT)):
                tp = mps.tile([128, 512], BF16, tag="misc")
                for c in range(2):
                    nc.tensor.transpose(tp[:, KCH * c:KCH * c + KCH],
                                        src[:, c, :], id_bf[:KCH, :KCH])
                nc.vector.tensor_copy(dst, tp[:, :KTOK])

            # ---- projections (feature-major outputs)
            qhT = sb.tile([128, QTOK], BF16, tag="qhT")
            for g in range(3):
                pp = mps.tile([128, 512], F32, tag="misc")
                nc.tensor.matmul(pp, wq_b, xhqT[:, 512 * g:512 * g + 512])
                nc.vector.tensor_scalar(
                    out=qhT[:, 512 * g:512 * g + 512], in0=pp,
                    scalar1=bwq[:, 0:1], scalar2=None,
                    op0=mybir.AluOpType.add)
            khT = sb.tile([128, KTOK], BF16, tag="khT")
            pp = mps.tile([128, 512], F32, tag="misc")
            nc.tensor.matmul(pp[:, :KTOK], wk_b, xhkT)
            nc.vector.tensor_scalar(
                out=khT, in0=pp[:, :KTOK], scalar1=bwk[:, 0:1], scalar2=None,
                op0=mybir.AluOpType.add)
            vh = sb.tile([KCH, 2, D], BF16, tag="vh")
            for c in range(2):
                pp = mps.tile([128, 512], F32, tag="misc")
                nc.tensor.matmul(pp[:KCH, :D],
                                 xhvT[:, KCH * c:KCH * c + KCH], wv_b)
                nc.vector.tensor_copy(vh[:, c, :], pp[:KCH, :D])

            # ---- scores + exp, per (head, k-chunk); dotT layout [k, q]
            ets = []
            for h in range(HEADS):
                for c in range(2):
                    dps = dotp.tile([128, QTOK], F32, tag="dot")
                    for b in range(QTOK // QB):
                        nc.tensor.matmul(
                            dps[:KCH, QB * b:QB * b + QB],
                            khT[32 * h:32 * h + 32, KCH * c:KCH * c + KCH],
                            qhT[32 * h:32 * h + 32, QB * b:QB * b + QB],
                            tile_position=(32 * h, 0))
                    et = etp.tile([128, QTOK], BF16, tag="et")
                    nc.scalar.activation(out=et[:KCH, :], in_=dps[:KCH, :],
                                         func=AF.Exp, scale=SCALE)
                    ets.append(et)

            # ---- denominator + att@v + normalize, per q-block
            recipT = sb.tile([128, QTOK], F32, tag="recipT")
            aT = sb.tile([128, QTOK], BF16, tag="aT")
            for b in range(QTOK // QB):
                den = mps.tile([128, QB], F32, tag="misc")
                for h in range(HEADS):
                    for c in range(2):
                        et = ets[2 * h + c]
                        nc.tensor.matmul(
                            den[32 * h:32 * h + 32, :],
                            ones_bf[:KCH, :], et[:KCH, QB * b:QB * b + QB],
                            start=(c == 0), stop=(c == 1),
                            tile_position=(0, 32 * h))
                nc.vector.reciprocal(
                    out=recipT[:, QB * b:QB * b + QB], in_=den)
                av = mps.tile([128, QB], F32, tag="misc")
                for h in range(HEADS):
                    for c in range(2):
                        et = ets[2 * h + c]
                        nc.tensor.matmul(
                            av[32 * h:32 * h + 32, :],
                            vh[:, c, 32 * h:32 * h + 32],
                            et[:KCH, QB * b:QB * b + QB],
                            start=(c == 0), stop=(c == 1),
                            tile_position=(0, 32 * h))
                nc.vector.tensor_tensor(
                    out=aT[:, QB * b:QB * b + QB], in0=av,
                    in1=recipT[:, QB * b:QB * b + QB],
                    op=mybir.AluOpType.mult)

            # ---- output projection with view-mean folded into PSUM
            zps = mps.tile([128, 256], F32, tag="misc")
            for n in range(NVIEW):
                nc.tensor.matmul(zps, wp_b, aT[:, 256 * n:256 * n + 256],
                                 start=(n == 0), stop=(n == NVIEW - 1))
            outT = sb.tile([128, 256], F32, tag="outT")
            nc.vector.tensor_scalar(
                out=outT, in0=zps, scalar1=1.0 / NVIEW, scalar2=bpe[:, 0:1],
                op0=mybir.AluOpType.mult, op1=mybir.AluOpType.add)

            # ---- back to token-major, add skip, store
            sk = sb.tile([128, 2, D], F32, tag="sk")
            nc.sync.dma_start(
                out=sk,
                in_=skip_t[w].rearrange("a b d -> (a b) d")
                             .rearrange("(c p) d -> p c d", p=128))
            fps = mps.tile([128, 256], F32, tag="misc")
            for i in range(2):
                nc.tensor.transpose(fps[:, 128 * i:128 * i + 128],
                                    outT[:, 128 * i:128 * i + 128], id_f32)
            res = sb.tile([128, 2, D], F32, tag="res")
            nc.vector.tensor_tensor(
                out=res, in0=fps.rearrange("p (c d) -> p c d", c=2), in1=sk,
                op=mybir.AluOpType.add)
            nc.sync.dma_start(
                out=out_t[w].rearrange("a b d -> (a b) d")
                            .rearrange("(c p) d -> p c d", p=128),
                in_=res)

    _split_waits(nc)
    return nc


_NC_CACHE = None


def _get_nc():
    global _NC_CACHE
    if _NC_CACHE is None:
        _NC_CACHE = build_nc()
    return _NC_CACHE


def kernel(**inputs):
    q = np.asarray(inputs["q"], dtype=np.float32)
    k = np.asarray(inputs["k"], dtype=np.float32)
    v = np.asarray(inputs["v"], dtype=np.float32)
    skip = np.asarray(inputs["skip"], dtype=np.float32)

    wstack = np.stack([inputs["Wq"], inputs["Wk"], inputs["Wv"], inputs["Wp"]]
                      ).astype(np.float32)
    pstack = np.stack([
        inputs["gq"], inputs["bq_ln"], inputs["gk"], inputs["bk_ln"],
        inputs["gv"], inputs["bv_ln"], inputs["bq"], inputs["bk"],
        inputs["bv"], inputs["bp"]], axis=1).astype(np.float32)

    nc = _get_nc()
    in_maps = []
    for c in range(8):
        in_maps.append({
            "q": np.ascontiguousarray(q[0, :, c]),
            "k": np.ascontiguousarray(k[0, :, c]),
            "v": np.ascontiguousarray(v[0, :, c]),
            "skip": np.ascontiguousarray(skip[0, c]),
            "wstack": wstack,
            "pstack": pstack,
        })
    import os
    trace = bool(os.environ.get("KERNEL_TRACE"))
    res = run_bass_kernel_spmd(nc, in_maps, core_ids=list(range(8)),
                               trace=trace)
    kernel.last_result = res
    out = np.stack([res.results[c]["out"] for c in range(8)], axis=0)
    return out[None]  # (1, 8, 8, 16, 16, 128)

